# revision 3
# baseline (speedup 1.0000x reference)
"""Trainium2 Bass kernel for nn_AttentionWithVQ (B=4, N=2048, DIM=512, H=8,
depthwise-conv positional term, softmax attention, output projection).

Sharding: data-parallel over B (4 batches x 2 core-groups) and tensor-parallel
over heads (4 heads per core) -> 8 cores, fully independent per core except a
final partial-sum over the two head-groups of each batch, done on host at
gather time (the output projection contracts over heads).

Core algorithmic fusion: the score matrix
    S = 0.5*(scale * q @ k^T + scale * conv1(m) @ conv2(s)^T)
is ONE matmul over a concatenated 128-feature axis:
    S = Qp^T @ Kp,  Qp = [q*scale*0.5 ; conv1(m)*scale*0.5], Kp = [k ; conv2(s)]
which exactly fills the 128x128 PE array contraction dim.

Softmax denominators come for free by appending a ones-column to V
(attn@[V|1] yields the row-sums of exp(S) in the last output row); exp() is
numerically safe without max-subtraction for this problem's score magnitudes
(|S| < ~1 given the 0.02-scaled weights).

Partition alignment: compute engines are lane-locked (PSUM partition p ->
SBUF partition p), so per-head feature layouts alternate by head parity
(even heads [qk;conv], odd heads [conv;qk]) making every PSUM->SBUF copy
partition-aligned; the few genuinely shifting copies (odd-head attention
outputs, denominator rows) go through DMA, which can move partitions freely.
All permutation bookkeeping is done host-side in numpy when preparing
per-core inputs.
"""

import os
import sys

sys.path.insert(0, "/opt/trn_rl_repo")

import numpy as np

# ---------------------------------------------------------------- constants
B, N, DIM, HEAD, VQE_K = 4, 2048, 512, 8, 3
Dh = DIM // HEAD            # 64
HPC = HEAD // 2             # heads per core (8 cores = 4 batch * 2 groups)
P = 128
FB = 512                    # free-dim block (one fp32 PSUM bank)
NQB = N // FB               # 4
NKB = N // P                # 16
SCALE_Q = Dh ** -0.5 * 0.5  # folds the 0.5 score scale into the q/conv1 side

# which matmul groups run in float32r (1 cyc/row) vs float32 (4 cyc/row)
_DEFAULT_CFG = {"qkv": "f32", "attn": "f32", "proj": "f32"}

_CACHE = {}


# ---------------------------------------------------------------- host prep
def _host_prep(core, inp):
    """Build the per-core input arrays (sharding + layout permutations)."""
    b, g = core // 2, core % 2
    f32 = np.float32
    x, m, s = inp["x"], inp["m"], inp["s"]
    qkv_w, qkv_b = inp["qkv_w"], inp["qkv_b"]
    proj_w = inp["proj_w"]
    p1w = inp["pe1_w"].reshape(HEAD, VQE_K)
    p2w = inp["pe2_w"].reshape(HEAD, VQE_K)
    pe1_b, pe2_b = inp["pe1_b"], inp["pe2_b"]

    d = {}
    d["xt"] = np.ascontiguousarray(x[b].T, dtype=f32)  # [512, 2048]

    # m/s transposed, tile t rows = [head(2t+1) feats ; head(2t) feats]
    mt = np.empty((256, N), f32)
    st = np.empty((256, N), f32)
    mcw = np.zeros((128, 8), f32)
    scw = np.zeros((128, 8), f32)
    for t in range(2):
        h_lo, h_hi = g * 4 + 2 * t + 1, g * 4 + 2 * t
        mt[t * 128:t * 128 + 64] = m[b][:, h_lo * 64:(h_lo + 1) * 64].T
        mt[t * 128 + 64:t * 128 + 128] = m[b][:, h_hi * 64:(h_hi + 1) * 64].T
        st[t * 128:t * 128 + 64] = s[b][:, h_lo * 64:(h_lo + 1) * 64].T
        st[t * 128 + 64:t * 128 + 128] = s[b][:, h_hi * 64:(h_hi + 1) * 64].T
        for p in range(128):
            h = g * 4 + 2 * t + (1 if p < 64 else 0)
            mcw[p, 4 * t:4 * t + 3] = p1w[h] * SCALE_Q
            scw[p, 4 * t:4 * t + 3] = p2w[h]
            mcw[p, 4 * t + 3] = pe1_b[h] * SCALE_Q
            scw[p, 4 * t + 3] = pe2_b[h]
    d["mt"], d["st"], d["mcw"], d["scw"] = mt, st, mcw, scw

    # q/k projection weights: chunk ch=(t, q|k) = [even-head rows; odd-head rows]
    wqk_f = np.empty((512, DIM), f32)
    qkb = np.zeros((128, 4), f32)
    for t in range(2):
        for j in range(2):  # 0=q, 1=k
            ch = 2 * t + j
            h_e, h_o = g * 4 + 2 * t, g * 4 + 2 * t + 1
            base = j * DIM
            wqk_f[ch * 128:ch * 128 + 64] = qkv_w[base + h_e * 64:base + (h_e + 1) * 64]
            wqk_f[ch * 128 + 64:(ch + 1) * 128] = qkv_w[base + h_o * 64:base + (h_o + 1) * 64]
            qkb[0:64, ch] = qkv_b[base + h_e * 64:base + (h_e + 1) * 64]
            qkb[64:128, ch] = qkv_b[base + h_o * 64:base + (h_o + 1) * 64]
            if j == 0:
                wqk_f[ch * 128:(ch + 1) * 128] *= SCALE_Q
                qkb[:, ch] *= SCALE_Q
    d["wqk"] = np.ascontiguousarray(wqk_f.T)  # [c=512, f=512]
    d["qkb"] = qkb

    d["wv"] = np.ascontiguousarray(
        qkv_w[2 * DIM + g * 256:2 * DIM + (g + 1) * 256].T, dtype=f32)  # [512, 256]

    # proj rows / v-bias / one-hot broadcast matrix in aT partition order:
    # aT tile t partition p -> head 2t+(p>=64), d=p%64
    pjt = np.empty((256, DIM), f32)
    vbv = np.empty((256,), f32)
    e4 = np.zeros((4, 256), f32)
    for t in range(2):
        for p in range(128):
            h_l = 2 * t + (1 if p >= 64 else 0)
            h = g * 4 + h_l
            pjt[t * 128 + p] = proj_w[:, h * 64 + (p % 64)]
            vbv[t * 128 + p] = qkv_b[2 * DIM + h * 64 + (p % 64)]
            e4[h_l, t * 128 + p] = 1.0
    d["pjt"], d["e4"] = pjt, e4
    d["vbv"] = np.ascontiguousarray(vbv.reshape(2, 128).T)  # [128, 2]
    return d


# ------------------------------------------------------------- device build
def _emit(tc, nc, io, cfg):
    from contextlib import ExitStack

    from concourse import mybir

    dt = mybir.dt
    f32 = dt.float32
    AF = mybir.ActivationFunctionType
    ALU = mybir.AluOpType

    def mm(ap, group):
        return ap.bitcast(dt.float32r) if cfg[group] == "f32r" else ap

    with ExitStack() as ctx:
        persist = ctx.enter_context(tc.tile_pool(name="persist", bufs=1))

        # ---- persistent weight / activation tiles
        wqk_sb, wv_sb, xt_sb = [], [], []
        QP, KP, v_sb, aT, pjt_sb = [], [], [], [], []
        for c in range(4):
            w = persist.tile([128, 512], f32, name=f"wqk{c}", tag=f"wqk{c}")
            nc.sync.dma_start(w[:], io["wqk"][c * 128:(c + 1) * 128, :])
            wqk_sb.append(w)
            w = persist.tile([128, 256], f32, name=f"wv{c}", tag=f"wv{c}")
            nc.sync.dma_start(w[:], io["wv"][c * 128:(c + 1) * 128, :])
            wv_sb.append(w)
        for f in range(2):
            w = persist.tile([128, 512], f32, name=f"pjt{f}", tag=f"pjt{f}")
            nc.sync.dma_start(w[:], io["pjt"][f * 128:(f + 1) * 128, :])
            pjt_sb.append(w)
        e4_sb = persist.tile([4, 256], f32, name="e4", tag="e4")
        nc.sync.dma_start(e4_sb[:], io["e4"][:, :])
        mcw_sb = persist.tile([128, 8], f32, name="mcw", tag="mcw")
        nc.sync.dma_start(mcw_sb[:], io["mcw"][:, :])
        scw_sb = persist.tile([128, 8], f32, name="scw", tag="scw")
        nc.sync.dma_start(scw_sb[:], io["scw"][:, :])
        qkb_sb = persist.tile([128, 4], f32, name="qkb", tag="qkb")
        nc.sync.dma_start(qkb_sb[:], io["qkb"][:, :])
        vbv_sb = persist.tile([128, 2], f32, name="vbv", tag="vbv")
        nc.sync.dma_start(vbv_sb[:], io["vbv"][:, :])

        for h in range(HPC):
            QP.append(persist.tile([128, N], f32, name=f"QP{h}", tag=f"QP{h}"))
            KP.append(persist.tile([128, N], f32, name=f"KP{h}", tag=f"KP{h}"))
        for blk in range(NKB):
            v_sb.append(persist.tile([128, HPC * 65], f32, name=f"vsb{blk}",
                                     tag=f"vsb{blk}"))
        for t in range(2):
            aT.append(persist.tile([128, N], f32, name=f"aT{t}", tag=f"aT{t}"))
        denoms = persist.tile([4, N], f32, name="denoms", tag="denoms")

        # ---- phases 2+3: qkv projections (x^T resident only here)
        with tc.tile_pool(name="xtp", bufs=1) as xtp:
            for c in range(4):
                xt = xtp.tile([128, N], f32, name=f"xt{c}", tag=f"xt{c}")
                nc.sync.dma_start(xt[:], io["xt"][c * 128:(c + 1) * 128, :])
                xt_sb.append(xt)

            with tc.tile_pool(name="ps_qk", bufs=4, space="PSUM") as ps_qkp:
                for t in range(2):
                    for j in range(2):
                        ch = 2 * t + j
                        dst = QP if j == 0 else KP
                        for q4 in range(NQB):
                            qs = slice(q4 * FB, (q4 + 1) * FB)
                            ps = ps_qkp.tile([128, FB], f32, name="psqk", tag="psqk")
                            for c in range(4):
                                nc.tensor.matmul(
                                    ps[:],
                                    mm(wqk_sb[c][:, ch * 128:(ch + 1) * 128], "qkv"),
                                    mm(xt_sb[c][:, qs], "qkv"),
                                    start=(c == 0), stop=(c == 3))
                            nc.scalar.activation(
                                dst[2 * t][0:64, qs], ps[0:64, :], AF.Identity,
                                bias=qkb_sb[0:64, ch:ch + 1])
                            nc.scalar.activation(
                                dst[2 * t + 1][64:128, qs], ps[64:128, :], AF.Identity,
                                bias=qkb_sb[64:128, ch:ch + 1])

            with tc.tile_pool(name="ps_v", bufs=4, space="PSUM") as ps_vp:
                for blk in range(NKB):
                    bs = slice(blk * 128, (blk + 1) * 128)
                    ps = ps_vp.tile([128, 256], f32, name="psv", tag="psv")
                    for c in range(4):
                        nc.tensor.matmul(ps[:], mm(xt_sb[c][:, bs], "qkv"),
                                         mm(wv_sb[c][:], "qkv"),
                                         start=(c == 0), stop=(c == 3))
                    v3 = v_sb[blk].rearrange("p (h f) -> p h f", h=HPC)
                    nc.vector.tensor_copy(v3[:, :, 0:64],
                                          ps.rearrange("p (h f) -> p h f", h=HPC))
                    nc.vector.memset(v3[:, :, 64:65], 1.0)

        # ---- phase 4: depthwise convs (DVE; runs concurrent with PE phases)
        with tc.tile_pool(name="conv", bufs=2) as convp:
            for src, wv_, dst in (("mt", mcw_sb, QP), ("st", scw_sb, KP)):
                for t in range(2):
                    xin = convp.tile([128, N], f32, name=f"ci_{src}{t}", tag="cin")
                    nc.sync.dma_start(xin[:], io[src][t * 128:(t + 1) * 128, :])
                    y = convp.tile([128, N], f32, name=f"cy_{src}{t}", tag="cy",
                                   bufs=1)
                    w0, w1, w2, cb = (wv_[:, 4 * t + k:4 * t + k + 1]
                                      for k in range(4))
                    nc.vector.tensor_scalar(y[:], xin[:], w1, cb,
                                            ALU.mult, ALU.add)
                    nc.vector.scalar_tensor_tensor(
                        y[:, 1:], xin[:, :N - 1], w0, y[:, 1:],
                        ALU.mult, ALU.add)
                    nc.vector.scalar_tensor_tensor(
                        y[:, :N - 1], xin[:, 1:], w2, y[:, :N - 1],
                        ALU.mult, ALU.add)
                    nc.vector.tensor_copy(dst[2 * t + 1][0:64, :], y[0:64, :])
                    nc.vector.tensor_copy(dst[2 * t][64:128, :], y[64:128, :])

        # ---- phase 5: attention (fused score matmul + exp + attn@[V|1])
        with tc.tile_pool(name="ps_s", bufs=2, space="PSUM") as ps_sp, \
                tc.tile_pool(name="ps_o", bufs=1, space="PSUM") as ps_op, \
                tc.tile_pool(name="esbp", bufs=3) as esbp, \
                tc.tile_pool(name="stg", bufs=2) as stgp:
            for h in range(HPC):
                o_ps = ps_op.tile([65, N], f32, name=f"ops{h}", tag="ops")
                vcols = slice(h * 65, (h + 1) * 65)
                for nk in range(NKB):
                    ks = slice(nk * 128, (nk + 1) * 128)
                    for q4 in range(NQB):
                        qs = slice(q4 * FB, (q4 + 1) * FB)
                        s_ps = ps_sp.tile([128, FB], f32, name="sps", tag="sps")
                        nc.tensor.matmul(s_ps[:], mm(KP[h][:, ks], "attn"),
                                         mm(QP[h][:, qs], "attn"),
                                         start=True, stop=True)
                        e_sb = esbp.tile([128, FB], f32, name="esb", tag="esb")
                        nc.scalar.activation(e_sb[:], s_ps[:], AF.Exp)
                        nc.tensor.matmul(o_ps[:, qs], mm(v_sb[nk][:, vcols], "attn"),
                                         mm(e_sb[:], "attn"),
                                         start=(nk == 0), stop=(nk == NKB - 1))
                t, odd = h // 2, h % 2
                # DMA cannot read PSUM, and compute engines are lane-locked,
                # so partition-shifting copies stage through SBUF first.
                stg = stgp.tile([65, N], f32, name=f"stg{h}", tag="stg")
                if odd:
                    nc.vector.tensor_copy(stg[0:65, :], o_ps[0:65, :])
                    nc.sync.dma_start(aT[t][64:128, :], stg[0:64, :])
                else:
                    nc.vector.tensor_copy(aT[t][0:64, :], o_ps[0:64, :])
                    nc.vector.tensor_copy(stg[64:65, :], o_ps[64:65, :])
                # denominator row (stage partition 64 -> denoms partition h)
                nc.sync.dma_start(denoms[h:h + 1, :], stg[64:65, :])
            nc.vector.reciprocal(denoms[:], denoms[:])

        # ---- phase 6: normalize by softmax denominators (+ v bias)
        with tc.tile_pool(name="ps_bc", bufs=2, space="PSUM") as ps_bcp:
            for t in range(2):
                for q4 in range(NQB):
                    qs = slice(q4 * FB, (q4 + 1) * FB)
                    bc = ps_bcp.tile([128, FB], f32, name="bc", tag="bc")
                    # exact broadcast: one-hot fp32 matmul
                    nc.tensor.matmul(bc[:], e4_sb[:, t * 128:(t + 1) * 128],
                                     denoms[:, qs], start=True, stop=True)
                    nc.vector.tensor_mul(aT[t][:, qs], aT[t][:, qs], bc[:])
                nc.vector.tensor_scalar_add(aT[t][:], aT[t][:],
                                            vbv_sb[:, t:t + 1])

        # ---- phase 7: output projection (partial over this core's heads)
        with tc.tile_pool(name="ps_pj", bufs=3, space="PSUM") as ps_pjp, \
                tc.tile_pool(name="osbp", bufs=3) as osbp:
            for blk in range(NKB):
                bs = slice(blk * 128, (blk + 1) * 128)
                pj = ps_pjp.tile([128, FB], f32, name="pj", tag="pj")
                for f in range(2):
                    nc.tensor.matmul(pj[:], mm(aT[f][:, bs], "proj"),
                                     mm(pjt_sb[f][:], "proj"),
                                     start=(f == 0), stop=(f == 1))
                ob = osbp.tile([128, FB], f32, name="ob", tag="ob")
                nc.scalar.copy(ob[:], pj[:])
                nc.sync.dma_start(io["out"][bs, :], ob[:])


def _build(cfg_key):
    from concourse import bacc, mybir, tile

    cfg = dict(cfg_key)
    dt = mybir.dt
    nc = bacc.Bacc("TRN2", target_bir_lowering=False, debug=False,
                   num_devices=8)
    shapes = {
        "xt": [DIM, N], "mt": [256, N], "st": [256, N],
        "wqk": [DIM, 512], "wv": [DIM, 256], "pjt": [256, DIM],
        "e4": [4, 256], "mcw": [128, 8], "scw": [128, 8],
        "qkb": [128, 4], "vbv": [128, 2],
    }
    io = {}
    for name, shape in shapes.items():
        io[name] = nc.dram_tensor(name, shape, dt.float32,
                                  kind="ExternalInput").ap()
    io["out"] = nc.dram_tensor("out", [N, DIM], dt.float32,
                               kind="ExternalOutput").ap()
    with tile.TileContext(nc) as tc:
        _emit(tc, nc, io, cfg)
    nc.compile()
    return nc


def _get_program(cfg):
    key = tuple(sorted(cfg.items()))
    if key not in _CACHE:
        _CACHE[key] = _build(key)
    return _CACHE[key]


# ------------------------------------------------------------------ wrapper
def kernel(_cfg=None, _want_results=False, **inputs):
    from concourse.bass_utils import run_bass_kernel_spmd

    cfg = dict(_DEFAULT_CFG)
    if _cfg:
        cfg.update(_cfg)
    env_cfg = os.environ.get("BASSKERN_CFG")
    if env_cfg:  # e.g. "attn=f32r,qkv=f32r"
        for kv in env_cfg.split(","):
            k, v = kv.split("=")
            cfg[k] = v

    inputs = {k: np.asarray(v, dtype=np.float32) for k, v in inputs.items()}
    nc = _get_program(cfg)
    in_maps = [_host_prep(core, inputs) for core in range(8)]
    res = run_bass_kernel_spmd(nc, in_maps, list(range(8)))

    out = np.empty((B, N, DIM), np.float32)
    pb = inputs["proj_b"]
    for b in range(B):
        out[b] = res.results[2 * b]["out"] + res.results[2 * b + 1]["out"] + pb
    if _want_results:
        return out, res
    return out
